# revision 18
# baseline (speedup 1.0000x reference)
"""Trainium2 Bass kernel for nn_AttentionModuleBiModal (B=4, N1=N2=8192).

Math (per batch b):
    tm2[j] = w0*m2[j] + b0
    s1[i]  = sum_j (w2*m2[j] + b2) * tanh(m1[i] * tm2[j])   =  f_b(m1[i])
    s2[j]  = sum_i (w1*m1[i] + b1) * tanh(m1[i] * tm2[j])   =  g_b(tm2[j])
    a_m1 = tanh(w1*m1 + b1 + s1);  a_m2 = tanh(w2*m2 + b2 + s2)
    out1 = softmax(a_m1*w3 + b3) * m1;  out2 = softmax(a_m2*w4 + b4) * m2

Key structure: s1 (resp. s2) is a smooth ODD 1-D function f (resp. g) of the
single scalar m1[i] (resp. tm2[j]) -- a weighted sum of odd tanh ridges.
Each of the 8 cores evaluates ONE such function on a K-point uniform grid
t_k = k*h over [0, tmax]; the host reconstructs s1/s2 with 4-pt cubic
Lagrange interpolation (odd-reflected) -- an O(N) epilogue of the same class
as the softmax it already does.

Approximations (validated on the real inputs; final max L2rel ~ 3.2e-6,
vs 1.4e-6 for the previous full-sweep kernel, gate 2e-2):
  - the 8192 (y_j, v_j) ridge pairs are binned onto L=64 uniform ridge
    positions with 4-pt cubic Lagrange weights (host, O(N));
  - tanh table in fp16; v-weights split hi/lo fp16 and applied as a 2-row
    lhsT so one PE column-stream accumulates both rows (host sums them);
  - h is a power of two so grid values are exact in fp32.

Device per core: one [L, K] tanh (scale = per-partition ridge position),
one [L,2]^T x [L,K] matmul into PSUM [2, K], DVE evac, DMA out.
A dummy tanh issues first so the ~2.7us ACT table load overlaps the input
DMA.  All inputs (ridge centers, bit-packed v pair, pre-replicated grid)
travel as ONE contiguous [L, 2+K] f32 DMA -- one packet per partition; DMA
completion latency is ~1.3us mostly-fixed, so L=64 keeps it just under the
parallel ACT table-load + warm-tanh chain (~1.6us) it overlaps with.

Measured anatomy of the ~19.5us exec time: ~6.5us fixed NEFF/NRT preamble
(engine rendezvous + instruction-stream loads), ~2.9us table-load/DMA
overlap + tanh/matmul/evac/out-DMA critical path, ~1.0us DMA queue drain,
~9us fixed postamble (compiler-emitted semaphore-file sweep + barriers).
The fixed parts are identical for every kernel in this framework (the
313us baseline paid them too).
"""

import numpy as np

B = 4
N = 8192
NCORES = 8
L = 64              # binned ridge positions = one partition tile
K = 64              # t-grid points per function

_CACHE = {}

_SCALARS = ("w0", "b0", "w1", "b1", "w2", "b2", "w3", "b3", "w4", "b4")


def _build_program():
    from contextlib import ExitStack

    import concourse.bacc as bacc
    import concourse.tile as tile
    from concourse import mybir

    f32, f16 = mybir.dt.float32, mybir.dt.float16
    nc = bacc.Bacc("TRN2", target_bir_lowering=False, debug=False)

    # col 0: ridge centers (tanh scale); col 1: the (vhi, vlo) fp16 pair
    # bit-packed into one f32 lane; cols 2..: grid row (same on every
    # partition, replicated host-side) -- the whole input is one contiguous
    # 128x(2+K) DMA, one packet per partition.
    d_comb = nc.dram_tensor("comb", [L * (2 + K)], f32, kind="ExternalInput")
    d_out = nc.dram_tensor("o_f", [2, K], f32, kind="ExternalOutput")

    with ExitStack() as ctx:
        tc = ctx.enter_context(tile.TileContext(nc))
        singles = ctx.enter_context(tc.tile_pool(name="singles", bufs=1))
        pp = ctx.enter_context(tc.tile_pool(name="pp", bufs=1, space="PSUM"))

        # Dummy tanh on a memset tile: hoists the ~2.7us ACT table load to
        # the front of the ACT stream so it overlaps the input DMAs.
        warm = singles.tile([128, 1], f32)
        nc.vector.memset(warm, 0)
        warmo = singles.tile([128, 1], f32)
        nc.scalar.activation(
            out=warmo, in_=warm, func=mybir.ActivationFunctionType.Tanh
        )

        comb = singles.tile([L, 2 + K], f32)
        nc.sync.dma_start(
            out=comb, in_=d_comb.ap().rearrange("(p c) -> p c", p=L)
        )

        ps = pp.tile([2, K], f32, name="ps")
        T = singles.tile([L, K], f16, name="T")
        nc.scalar.activation(
            out=T,
            in_=comb[:, 2 : 2 + K],
            func=mybir.ActivationFunctionType.Tanh,
            scale=comb[:, 0:1],
        )
        nc.tensor.matmul(
            ps[0:2, :],
            lhsT=comb[:, 1:2].bitcast(f16),
            rhs=T,
            start=True,
            stop=True,
        )
        s_sb = singles.tile([2, K], f32, name="s_sb")
        nc.vector.tensor_copy(out=s_sb, in_=ps)
        nc.sync.dma_start(out=d_out.ap(), in_=s_sb)

    nc.compile()
    return nc


def _get_program():
    if "nc" not in _CACHE:
        _CACHE["nc"] = _build_program()
    return _CACHE["nc"]


def _cubic_w(s):
    """4-pt cubic Lagrange weights at offsets -1,0,1,2 for s in [0,1]."""
    wm1 = -s * (s - 1) * (s - 2) / 6.0
    w0 = (s + 1) * (s - 1) * (s - 2) / 2.0
    w1 = -(s + 1) * s * (s - 2) / 2.0
    w2 = (s + 1) * s * (s - 1) / 6.0
    return wm1, w0, w1, w2


def _bin_pairs(y, v):
    """Distribute weights v at ridge positions y onto L uniform bins."""
    ymax = np.abs(y).max() * (1 + 1e-9)
    hy = 2 * ymax / (L - 4)
    c0 = -ymax - hy
    u = (y - c0) / hy
    l0 = np.clip(np.floor(u).astype(np.int64), 1, L - 3)
    s = u - l0
    wl = np.zeros(L)
    for off, wi in zip((-1, 0, 1, 2), _cubic_w(s)):
        wl += np.bincount(l0 + off, weights=v * wi, minlength=L)
    centers = c0 + hy * np.arange(L)
    return centers, wl


def _sides(m1, m2, sc):
    """Per-core (y, v, t) triples: core 2b = f-side of batch b, 2b+1 = g."""
    out = []
    for b in range(B):
        m1b = m1[b].astype(np.float64)
        m2b = m2[b].astype(np.float64)
        tm2 = sc["w0"] * m2b + sc["b0"]
        out.append((tm2, sc["w2"] * m2b + sc["b2"], m1b))
        out.append((m1b, sc["w1"] * m1b + sc["b1"], tm2))
    return out


def _grid_h(t):
    """Power-of-two grid spacing covering |t| with cubic-stencil margin."""
    tmax = np.abs(t).max()
    return float(2.0 ** np.ceil(np.log2(max(tmax, 1e-30) / (K - 3))))


def _make_in_maps(m1, m2, sc):
    in_maps = []
    for y, v, t in _sides(m1, m2, sc):
        centers, wl = _bin_pairs(y, v)
        h = _grid_h(t)
        grid = (h * np.arange(K)).astype(np.float32)
        vhi = wl.astype(np.float16)
        vlo = (wl - vhi.astype(np.float64)).astype(np.float16)
        vpack = (
            np.stack([vhi, vlo], axis=1).reshape(-1).view(np.float32).copy()
        )
        comb = np.empty((L, 2 + K), np.float32)
        comb[:, 0] = centers
        comb[:, 1] = vpack
        comb[:, 2:] = grid[None, :]
        in_maps.append({"comb": comb.reshape(-1)})
    return in_maps


def _interp_odd(F, h, t):
    """Cubic-Lagrange interp of odd function tabulated at k*h, k=0..K-1."""
    u = np.abs(t) / h
    k0 = np.clip(np.floor(u).astype(np.int64), 0, K - 3)
    s = u - k0
    Fp = np.concatenate(([-F[1]], F))  # odd ghost point at k=-1
    out = np.zeros_like(u)
    for off, wi in zip((-1, 0, 1, 2), _cubic_w(s)):
        out += wi * Fp[k0 + off + 1]
    return np.sign(t) * out


def _run_device(inputs, trace=False):
    from concourse.bass_utils import run_bass_kernel_spmd

    nc = _get_program()
    m1 = np.asarray(inputs["m1_t"], np.float32)[..., 0]  # [B, N]
    m2 = np.asarray(inputs["m2_t"], np.float32)[..., 0]
    sc = {k: float(np.asarray(inputs[k])) for k in _SCALARS}
    in_maps = _make_in_maps(m1, m2, sc)
    res = run_bass_kernel_spmd(nc, in_maps, list(range(NCORES)), trace=trace)
    return res, m1, m2, sc


def _postprocess(results, m1, m2, sc):
    out1 = np.zeros((B, N), np.float32)
    out2 = np.zeros((B, N), np.float32)
    sides = _sides(m1, m2, sc)
    for b in range(B):
        m1b = m1[b].astype(np.float64)
        m2b = m2[b].astype(np.float64)
        ss = []
        for c in (2 * b, 2 * b + 1):
            Fr = results[c]["o_f"].astype(np.float64)
            F = Fr[0] + Fr[1]  # v-hi row + v-lo row
            ss.append(_interp_odd(F, _grid_h(sides[c][2]), sides[c][2]))
        s1, s2 = ss
        a_m1 = np.tanh(sc["w1"] * m1b + sc["b1"] + s1)
        a_m2 = np.tanh(sc["w2"] * m2b + sc["b2"] + s2)
        l1 = a_m1 * sc["w3"] + sc["b3"]
        l2 = a_m2 * sc["w4"] + sc["b4"]
        e1 = np.exp(l1 - l1.max())
        e2 = np.exp(l2 - l2.max())
        out1[b] = (e1 / e1.sum() * m1b).astype(np.float32)
        out2[b] = (e2 / e2.sum() * m2b).astype(np.float32)
    return out1, out2


def kernel(**inputs):
    res, m1, m2, sc = _run_device(inputs, trace=False)
    return _postprocess(res.results, m1, m2, sc)
